# revision 29
# baseline (speedup 1.0000x reference)
"""Trainium2 Bass kernel for nn_ConcatSquashLinearSA3.

Strategy: shard the N=2048 point dimension across 8 cores (256 rows each).
BatchNorm stats (per-n over (B, dim_out)) are fully core-local, so no
collectives are needed.  All on-chip activations live in a transposed
layout [dim_out on partitions, n on free]; the host transposes x during
sharding and un-transposes the output during the gather, so the device
never pays for layout changes.

All ctx-only terms (tiny functions of the [B,259] context vector,
identical on every core) are folded into host-side input prep: the FiLM
gate/bias vectors and, crucially, the per-batch mixing matrices
    M_b = W_tc @ (I - attention_b^T)          [dim_out x dim_out]
which combine the channel-attention apply, the residual subtract and the
trans_conv into ONE matmul chain on the device:
    t_b = M_b @ x1_b + b_tc.
M_b is exactly the same size as the attention matrix it replaces, so no
extra bytes move; the device drops two of its five elementwise streams
and a third of its matmuls.

Per core, per batch-pair j (b0=2j, b1=2j+1), on [128,1024] row-tiles
(cols = channel-block x batch-half x n):
  x1s  = bf16(W_layer.T-slices @ xT + b_layer)   (2 ACT casts from PSUM)
  tT   = bf16(M_b.T-slices @ x1s + b_tc)         (ACT + DVE casts)
  sq   = tT*tT                                   (GpSimd)
  stats: ones.T @ tT / ones.T @ sq accumulate into PSUM rows at
         col-groups 0/32 (concurrent 32x32 sub-array matmuls)
Then BN scale/shift are built once (rsqrt via a DVE pow so the ACT
Identity table never swaps), broadcast via K=1 matmuls, and pass 2 does
  bn2  = tT*scale + shift        (GpSimd mult, DVE add)
  xo   = relu(bn2) + x1s         (two single-pass DVE ops)
  out  = xo*gate + bias          (per-(channel-block,half) DVE/ACT)
streaming out as fp16, one [128,1024] DMA per pair.
"""

import os
import sys

sys.path.insert(0, "/opt/trn_rl_repo")

import numpy as np
import ml_dtypes

import concourse.bass as bass
import concourse.bacc as bacc
import concourse.mybir as mybir
import concourse.tile as tile
from concourse.bass_utils import run_bass_kernel_spmd

F32 = mybir.dt.float32
F32R = mybir.dt.float32r
BF16 = mybir.dt.bfloat16
FP16 = mybir.dt.float16

B, N, DIN, DOUT, DCTX = 32, 2048, 128, 256, 259
NCORES = 8
NLOC = N // NCORES          # 256 rows per core
PAIRS = B // 2              # 16 batch pairs
BN_EPS = 1e-5

Act = mybir.ActivationFunctionType
Alu = mybir.AluOpType

_cached = {}


def build_program(reps=1):
    nc = bacc.Bacc("TRN2", target_bir_lowering=False, debug=False,
                   num_devices=NCORES)

    # ---- DRAM I/O ----
    xT2 = nc.dram_tensor("xT2", [PAIRS, 128, 512],
                         mybir.dt.bfloat16, kind="ExternalInput").ap()
    wlt = nc.dram_tensor("WlT", [128, 256], mybir.dt.bfloat16,
                         kind="ExternalInput").ap()
    mtp = nc.dram_tensor("mP", [4, 128, 4096], mybir.dt.bfloat16,
                         kind="ExternalInput").ap()
    gh = nc.dram_tensor("gh", [2, 128, 64], F32, kind="ExternalInput").ap()
    bc4 = nc.dram_tensor("bc4", [128, 4], F32, kind="ExternalInput").ap()
    bnrow = nc.dram_tensor("bnrow", [1, 512], F32, kind="ExternalInput").ap()
    onesr = nc.dram_tensor("onesr", [1, 128], F32, kind="ExternalInput").ap()
    outT2 = nc.dram_tensor("outT2", [PAIRS, 128, 1024], FP16,
                           kind="ExternalOutput").ap()

    with tile.TileContext(nc) as tc:
        _emit(nc, tc, xT2, wlt, mtp, gh, bc4, bnrow, onesr, outT2,
              reps=reps)

    nc.compile()
    return nc


def _emit(nc, tc, xT2, wlt, mtp, gh, bc4, bnrow, onesr, outT2,
          reps=1):
    import contextlib
    ctx = contextlib.ExitStack()
    with ctx:
        if reps > 1:
            loop = ctx.enter_context(tc.For_i(0, reps, 1))
        p_const = ctx.enter_context(tc.tile_pool(name="const", bufs=1))
        p_xin = ctx.enter_context(tc.tile_pool(name="xin", bufs=4))
        p_stats = ctx.enter_context(tc.tile_pool(name="stats", bufs=2, space="PSUM"))
        p_x1s = ctx.enter_context(tc.tile_pool(name="x1s", bufs=16))
        p_tT = ctx.enter_context(tc.tile_pool(name="tT", bufs=16))
        p_att = ctx.enter_context(tc.tile_pool(name="att", bufs=4))
        p_small = ctx.enter_context(tc.tile_pool(name="small", bufs=1))

        # ---- load constants into SBUF ----
        # DMA priority: sync gets wlt + the first M chunk (needed by pair 0)
        # ahead of the xin stream; scalar gets the small consts + the rest.
        c_wlt = p_const.tile([128, 256], BF16, tag="wlt")
        nc.sync.dma_start(c_wlt[:], wlt[:])
        c_gh = []
        for ot in range(2):
            t = p_const.tile([128, 64], F32, tag=f"gh{ot}")
            nc.scalar.dma_start(t[:], gh[ot])
            c_gh.append(t)
        c_bc4 = p_const.tile([128, 4], F32, tag="bc4")
        nc.scalar.dma_start(c_bc4[:], bc4[:])
        c_bnr = p_const.tile([1, 512], F32, tag="bnr")
        nc.scalar.dma_start(c_bnr[:], bnrow[:])
        ones_row = p_const.tile([1, 128], F32R, tag="ones_row")
        nc.scalar.dma_start(ones_row[:], onesr[:].bitcast(F32R))
        ones16 = p_const.tile([128, 1], BF16, tag="ones16")
        nc.vector.memset(ones16[:], 1.0)
        ind32 = p_const.tile([64, 1], F32, tag="ind32")
        nc.vector.memset(ind32[:], 0.0)
        nc.vector.memset(ind32[32:33, :], 1.0)

        gate = [c_gh[ot][:, 0:32] for ot in range(2)]
        hb = [c_gh[ot][:, 32:64] for ot in range(2)]

        # first x tiles ahead of the 1MB M chunks in the sync DMA queue,
        # so pair 0's matmuls can start while M streams in
        xin_pre = {}
        for j in range(3):
            t = p_xin.tile([128, 512], BF16, tag="xin", name=f"xin{j}")
            nc.sync.dma_start(t[:], xT2[j])
            xin_pre[j] = t

        # M matrices, 16 (b,ot)-chunks per [128,4096] tile; the first
        # chunk is split across both DMA queues so pair 0's tT matmuls
        # unblock after 512KB instead of 1MB
        m4 = []
        for g in range(4):
            t = p_att.tile([128, 4096], BF16, tag="m4", name=f"m4_{g}")
            if g == 0:
                nc.sync.dma_start(t[:, 0:2048], mtp[0][:, 0:2048])
                nc.scalar.dma_start(t[:, 2048:4096], mtp[0][:, 2048:4096])
            else:
                nc.scalar.dma_start(t[:], mtp[g])
            m4.append(t)

        def m_slice(b, ot, et2):
            u = b * 2 + ot
            g, c = u // 16, u % 16
            return m4[g][:, c * 256 + et2 * 128: c * 256 + (et2 + 1) * 128]

        # ---- BN stats accumulators (held in PSUM all of pass 1) ----
        # sum lives at partition 0 (col-group 0), sumsq at partition 32
        # (col-group 1) so the two per-tile reduction matmuls run
        # concurrently in different 32x32 sub-arrays.
        st_both_t = p_stats.tile([64, 512], F32, tag="st_both", bufs=1)
        st_sum = st_both_t[0:1, :]
        st_sq = st_both_t[32:33, :]

        x1s_tiles = [None] * PAIRS
        tT_tiles = [None] * PAIRS

        # ======== PASS 1 ========
        with tc.tile_pool(name="big", bufs=4, space="PSUM") as p_big, \
             tc.tile_pool(name="sq", bufs=4) as p_sq, \
             tc.tile_pool(name="outp", bufs=2) as p_out:

            stats_qq = []
            STATS_LAG = 2

            def _flush_stats(jf, et2f):
                ttf, sqf = stats_qq[jf]
                first = (jf == 0 and et2f == 0)
                last = (jf == PAIRS - 1 and et2f == 1)
                sl = slice(et2f * 512, (et2f + 1) * 512)
                nc.tensor.matmul(st_sum, ones16[:], ttf[:, sl],
                                 start=first, stop=last,
                                 skip_group_check=True,
                                 tile_position=(0, 0))
                nc.tensor.matmul(st_sq, ones16[:], sqf[:, sl],
                                 start=first, stop=last,
                                 skip_group_check=True,
                                 tile_position=(0, 32))

            for j in range(PAIRS):
                if j in xin_pre:
                    xin = xin_pre[j]
                else:
                    xin = p_xin.tile([128, 512], BF16, tag="xin",
                                     name=f"xin{j}")
                    nc.sync.dma_start(xin[:], xT2[j])

                x1t_ps = []
                for ot in range(2):
                    ps = p_big.tile([128, 512], F32, tag="bigps",
                                    name=f"x1ps{j}_{ot}")
                    nc.tensor.matmul(
                        ps[:], c_wlt[:, ot * 128:(ot + 1) * 128],
                        xin[:], start=True, stop=True)
                    x1t_ps.append(ps)

                # x1s = bf16(x1T + b_layer), both channel blocks in one tile
                xs = p_x1s.tile([128, 1024], BF16, tag="x1s",
                                name=f"x1s{j}")
                x1s_tiles[j] = xs
                for ot in range(2):
                    nc.scalar.activation(xs[:, ot * 512:(ot + 1) * 512],
                                         x1t_ps[ot][:], Act.Identity,
                                         bias=c_bc4[:, ot:ot + 1])

                # tT = M_b @ x1s + b_tc   (one [128,1024] tile per pair)
                tt_ = p_tT.tile([128, 1024], BF16, tag="tT", name=f"tT{j}")
                tT_tiles[j] = tt_
                for et2 in range(2):
                    tps = p_big.tile([128, 512], F32, tag="bigps",
                                     name=f"tps{j}_{et2}")
                    for half in range(2):
                        b = 2 * j + half
                        for ot in range(2):
                            nc.tensor.matmul(
                                tps[:, half * 256:(half + 1) * 256],
                                m_slice(b, ot, et2),
                                xs[:, ot * 512 + half * 256:
                                   ot * 512 + (half + 1) * 256],
                                start=(ot == 0), stop=(ot == 1))
                    dst = tt_[:, et2 * 512:(et2 + 1) * 512]
                    if et2 == 0:
                        nc.vector.tensor_scalar(dst, tps[:],
                                                c_bc4[:, 2 + et2:3 + et2],
                                                None, Alu.add)
                    else:
                        nc.scalar.activation(dst, tps[:], Act.Identity,
                                             bias=c_bc4[:, 2 + et2:3 + et2])
                sq = p_sq.tile([128, 1024], BF16, tag="sq",
                               name=f"sq{j}", bufs=6)
                nc.gpsimd.tensor_tensor(sq[:], tt_[:], tt_[:], Alu.mult)
                stats_qq.append((tt_, sq))
                if j >= STATS_LAG:
                    for et2 in range(2):
                        _flush_stats(j - STATS_LAG, et2)
            for jj in range(PAIRS - STATS_LAG, PAIRS):
                for et2 in range(2):
                    _flush_stats(jj, et2)

            # ======== stats finalize ========
            st_sum_sb = p_small.tile([1, 512], F32, tag="st_sum_sb")
            nc.vector.tensor_scalar(st_sum_sb[:], st_sum, 0.0, None,
                                    Alu.add)
            # sumsq row sits at partition 32; engines are partition-locked,
            # so hop it to partition 0 with an indicator matmul (plain fp32)
            st_sq_sb32 = p_small.tile([64, 512], F32, tag="st_sq_sb32")
            nc.vector.memset(st_sq_sb32[:], 0.0)
            nc.vector.tensor_scalar(st_sq_sb32[32:33, :], st_sq, 0.0, None,
                                    Alu.add)
            sq_ps = p_big.tile([128, 512], F32, tag="bigps", name="sq_hop")
            nc.tensor.matmul(sq_ps[0:1, :], ind32[:], st_sq_sb32[:],
                             start=True, stop=True)
            st_sq_sb = p_small.tile([1, 512], F32, tag="st_sq_sb")
            nc.vector.tensor_scalar(st_sq_sb[:], sq_ps[0:1, :], 0.0, None,
                                    Alu.add)
            mean_r = p_small.tile([1, 256], F32, tag="mean")
            ex2_r = p_small.tile([1, 256], F32, tag="ex2")
            inv_n = 1.0 / (B * DOUT)
            tmp1 = p_small.tile([1, 256], F32, tag="tmp1")
            nc.vector.tensor_tensor(tmp1[:], st_sum_sb[0:1, 0:256],
                                    st_sum_sb[0:1, 256:512], Alu.add)
            nc.vector.tensor_scalar(mean_r[:], tmp1[:], inv_n, None, Alu.mult)
            tmp2 = p_small.tile([1, 256], F32, tag="tmp2")
            nc.vector.tensor_tensor(tmp2[:], st_sq_sb[0:1, 0:256],
                                    st_sq_sb[0:1, 256:512], Alu.add)
            nc.vector.tensor_scalar(ex2_r[:], tmp2[:], inv_n, None, Alu.mult)
            m2 = p_small.tile([1, 256], F32, tag="m2")
            nc.vector.tensor_tensor(m2[:], mean_r[:], mean_r[:], Alu.mult)
            var_r = p_small.tile([1, 256], F32, tag="var")
            nc.vector.tensor_tensor(var_r[:], ex2_r[:], m2[:], Alu.subtract)
            vpe = p_small.tile([1, 256], F32, tag="vpe")
            nc.vector.tensor_scalar(vpe[:], var_r[:], BN_EPS, None, Alu.add)
            std_r = p_small.tile([1, 256], F32, tag="std")
            nc.scalar.activation(std_r[:], vpe[:], Act.Sqrt)
            istd_r = p_small.tile([1, 256], F32, tag="istd")
            nc.vector.reciprocal(istd_r[:], std_r[:])
            scale_r = p_small.tile([1, 256], F32, tag="scl")
            nc.vector.tensor_tensor(scale_r[:], istd_r[:], c_bnr[0:1, 0:256],
                                    Alu.mult)
            ms = p_small.tile([1, 256], F32, tag="ms")
            nc.vector.tensor_tensor(ms[:], mean_r[:], scale_r[:], Alu.mult)
            shift_r = p_small.tile([1, 256], F32, tag="shf")
            nc.vector.tensor_tensor(shift_r[:], c_bnr[0:1, 256:512], ms[:],
                                    Alu.subtract)
            sc2 = p_small.tile([1, 512], F32R, tag="sc2")
            sh2 = p_small.tile([1, 512], F32R, tag="sh2")
            for hh in range(2):
                nc.vector.tensor_scalar(sc2[0:1, hh * 256:(hh + 1) * 256],
                                        scale_r[:], 0.0, None, Alu.add)
                nc.vector.tensor_scalar(sh2[0:1, hh * 256:(hh + 1) * 256],
                                        shift_r[:], 0.0, None, Alu.add)
            # broadcast to [128,1024] (duplicated across channel blocks)
            scale_bc = p_const.tile([128, 1024], BF16, tag="scale_bc")
            shift_bc = p_const.tile([128, 1024], BF16, tag="shift_bc")
            for src_t, dst in ((sc2, scale_bc), (sh2, shift_bc)):
                ps = p_big.tile([128, 512], F32, tag="bigps",
                                name=f"bcps_{dst.name}")
                nc.tensor.matmul(ps[:], ones_row[:], src_t[:],
                                 start=True, stop=True)
                nc.vector.tensor_scalar(dst[:, 0:512], ps[:], 0.0, None,
                                        Alu.add)
                nc.scalar.activation(dst[:, 512:1024], ps[:], Act.Identity)

            # ======== PASS 2 (no PSUM) ========
            for j in range(PAIRS):
                tt_ = tT_tiles[j]
                ob = p_out.tile([128, 1024], FP16, tag="ob", name=f"ob_{j}")
                bn1 = p_sq.tile([128, 1024], BF16, tag="bn1", bufs=2,
                                name=f"bn1_{j}")
                nc.vector.tensor_tensor(bn1[:], tt_[:], scale_bc[:],
                                        Alu.mult)
                bn2 = p_sq.tile([128, 1024], BF16, tag="bn2", bufs=2,
                                name=f"bn2_{j}")
                nc.vector.tensor_tensor(bn2[:], bn1[:], shift_bc[:], Alu.add)
                rl = p_sq.tile([128, 1024], BF16, tag="rl", bufs=2,
                               name=f"rl_{j}")
                nc.vector.tensor_scalar(rl[:], bn2[:], 0.0, None, Alu.max)
                xo = p_sq.tile([128, 1024], BF16, tag="xo", bufs=2,
                               name=f"xo_{j}")
                nc.vector.tensor_tensor(xo[:], rl[:], x1s_tiles[j][:],
                                        Alu.add)
                for ot in range(2):
                    for half in range(2):
                        b = 2 * j + half
                        cs = ot * 512 + half * 256
                        dst = ob[:, cs:cs + 256]
                        src = xo[:, cs:cs + 256]
                        nc.scalar.activation(
                            dst, src,
                            Act.Identity, scale=gate[ot][:, b:b + 1],
                            bias=hb[ot][:, b:b + 1])
                nc.sync.dma_start(outT2[j], ob[:])


def _prep_inputs(ctx, x, W_layer, b_layer, W_hbias, W_gate, b_gate,
                 W_k, W_v, W_tc, b_tc, bn_gamma, bn_beta):
    """Host-side shard + layout prep.  Returns list of 8 in_maps."""
    x = np.asarray(x, dtype=np.float32)
    ctx2 = np.asarray(ctx, dtype=np.float32).reshape(B, DCTX)
    shape = ctx2[:, :DCTX - 3]

    # ctx-only terms (identical on every core) on host, in f32
    z = ctx2 @ np.asarray(W_gate, np.float32).T + np.asarray(b_gate, np.float32)
    gate = 1.0 / (1.0 + np.exp(-z))                       # [B, DOUT]
    hbv = ctx2 @ np.asarray(W_hbias, np.float32).T        # [B, DOUT]
    kk = shape @ np.asarray(W_k, np.float32).T            # [B, DOUT]
    vv = shape @ np.asarray(W_v, np.float32).T            # [B, DOUT]
    # channel attention: softmax over e of k[o]*v[e], then / colsum over o
    energy = kk[:, :, None] * vv[:, None, :]              # [B, O, E]
    e = np.exp(energy - energy.max(axis=-1, keepdims=True))
    attn = e / e.sum(axis=-1, keepdims=True)
    attn = attn / (1e-9 + attn.sum(axis=1, keepdims=True))
    # fold attention + residual + trans_conv:  M_b = W_tc (I - A_b^T)
    Wtc = np.asarray(W_tc, np.float32)                    # [E2, O]
    Mt = np.empty((B, DOUT, DOUT), np.float32)            # lhsT: [b, o, f]
    eye = np.eye(DOUT, dtype=np.float32)
    for b in range(B):
        Mb = Wtc @ (eye - attn[b].T)                      # [f, o]
        Mt[b] = Mb.T                                      # [o, f]
    # pack for the device: [4, 128, 4096], 16 (b,ot)-chunks per row-tile
    mP = np.ascontiguousarray(
        Mt.reshape(B, 2, 128, DOUT).reshape(4, 16, 128, DOUT)
          .transpose(0, 2, 1, 3).reshape(4, 128, 4096)
    ).astype(ml_dtypes.bfloat16)

    ghm = np.zeros((2, 128, 64), np.float32)
    for ot in range(2):
        sl_o = slice(ot * 128, (ot + 1) * 128)
        ghm[ot, :, 0:32] = gate[:, sl_o].T
        ghm[ot, :, 32:64] = hbv[:, sl_o].T

    WlT = np.ascontiguousarray(
        np.asarray(W_layer, np.float32).T).astype(ml_dtypes.bfloat16)

    bc4 = np.zeros((128, 4), np.float32)
    bc4[:, 0:2] = np.asarray(b_layer, np.float32).reshape(2, 128).T
    bc4[:, 2:4] = np.asarray(b_tc, np.float32).reshape(2, 128).T

    gam = np.asarray(bn_gamma, np.float32)
    bet = np.asarray(bn_beta, np.float32)

    in_maps = []
    for c in range(NCORES):
        sl = slice(c * NLOC, (c + 1) * NLOC)
        xs = x[:, sl, :]                                   # [32, 256, 128]
        xT = xs.transpose(0, 2, 1)                         # [32, 128, 256]
        xT2 = np.ascontiguousarray(
            xT.reshape(PAIRS, 2, 128, 256).transpose(0, 2, 1, 3)
              .reshape(PAIRS, 128, 512)).astype(ml_dtypes.bfloat16)
        bnr = np.concatenate([gam[sl], bet[sl]]).reshape(1, 512)
        in_maps.append({
            "xT2": xT2, "WlT": WlT, "mP": mP,
            "gh": ghm, "bc4": bc4,
            "bnrow": np.ascontiguousarray(bnr.astype(np.float32)),
            "onesr": np.ones((1, 128), np.float32),
        })
    return in_maps


def kernel(**inputs):
    if "nc" not in _cached:
        _cached["nc"] = build_program()
    nc = _cached["nc"]
    in_maps = _prep_inputs(**inputs)
    res = run_bass_kernel_spmd(nc, in_maps, core_ids=list(range(NCORES)),
                               trace=bool(int(os.environ.get("KTRACE", "0"))))
    _cached["last_result"] = res
    out = np.empty((B, N, DOUT), np.float32)
    for c in range(NCORES):
        sl = slice(c * NLOC, (c + 1) * NLOC)
        r = np.asarray(res.results[c]["outT2"], dtype=np.float32)
        r = r.reshape(PAIRS, 128, 2, 2, 256)       # [j, p, ot, half, nl]
        r = r.transpose(0, 3, 4, 2, 1)             # [j, half, nl, ot, p]
        out[:, sl, :] = r.reshape(B, NLOC, DOUT)
    return out
